# revision 25
# baseline (speedup 1.0000x reference)
"""AcceptRejectPooling2D on 8 Trainium2 NeuronCores.

Reference semantics (per 2x2 window, stride 2, NHWC):
    r  = relu(x)
    s  = sum(r); ss = sum(r*r)
    out = ss / s   if s > 0 else 0

Sharding: pure data parallel over batch (64 -> 8 per core). Each core
processes x_local [8, 64, 64, 256] -> y_local [8, 32, 32, 256].

Layout per core: rows (b, h) of length W*C = 16384 floats. Output row
p = (b, ho) needs input rows 2p (even h) and 2p+1 (odd h). 256 output
rows = 2 partition groups of 128, each streamed as 4 tiles of 4096
row-columns ([128, 8192] f32 in SBUF).

Division via ACT tables: t = exp(-ln(s + eps) + ln(1024)) = 1024/s.
This keeps the whole reduction 16-bit on the DVE (2x packed mode for
every add and the final mul) and moves the divide to the scalar engine
whose Ln/Exp run at full rate from the same function table as
Relu/Square. ln output is fp16 (bf16's 8-bit mantissa costs ~3% in
exp(ln) round-trip at |ln|~9); t is bf16 (needs e+30 range for the
eps-guarded zero windows, where ss=0 makes out exactly 0). The output
is stored as fp16 scaled by 1024 (so values down to 6e-8 stay in fp16
normals) and unscaled on the host.

Engine balance per 4096-col chunk: ACT = relu + ln + exp + 1/8 of the
square; DVE = pair-adds + 7/8 of the square + final mul. Both ~12us.
A 2-deep software pipeline (ln/exp one chunk behind, mul/store two
behind) keeps the cross-engine handoffs off the critical path. First
and last tiles are sub-chunked so the pipe fills/drains fine-grained.
"""

import sys

if "/opt/trn_rl_repo" not in sys.path:
    sys.path.insert(0, "/opt/trn_rl_repo")

import math

import numpy as np

_B, _H, _W, _C = 8, 64, 64, 256  # per-core shard
_HO, _WO = _H // 2, _W // 2
_NP = 128                         # SBUF partitions
_F = 4096                         # row-columns per tile
_NG = (_B * _HO) // _NP           # partition groups (2)
_EPS = 1e-30
_LN_OSCALE = math.log(1024.0)     # out stored as 1024*(ss/s), fp16
_OSCALE_INV = 1.0 / 1024.0
# "lnexp": t = exp(-ln(s)+k) on ACT (faster, ~1.7e-2 rel err)
# "recip": t = 1024/s via DVE reciprocal_approx_fast (~1.3e-2 rel err)
_DIV_MODE = "recip"

_CACHE = {}


def _pin_act_table(bacc, mybir):
    """Route every activation to natural_log_exp_and_others (which holds
    Relu, Square, Ln AND Exp) so the kernel needs exactly one ACT
    function-table load. The compiler's per-instruction greedy set choice
    otherwise alternates sets (~2.7us reload each). Only the in-memory
    choice list is edited; set ids / loaded table bytes are unchanged.
    """
    if getattr(bacc, "_arp_act_pin", False):
        return
    AF = mybir.ActivationFunctionType
    pin = {AF.Relu, AF.Square, AF.Ln, AF.Exp}
    orig = bacc.get_activation_tables

    def pinned(arch):
        return {
            name: (fns if name == "natural_log_exp_and_others" else fns - pin)
            for name, fns in orig(arch).items()
        }

    bacc.get_activation_tables = pinned
    bacc._arp_act_pin = True


def _build_nc():
    import concourse.bacc as bacc
    import concourse.tile as tile
    from concourse import mybir

    _pin_act_table(bacc, mybir)
    nc = bacc.Bacc("TRN2", target_bir_lowering=False, debug=False, num_devices=8)
    f32 = mybir.dt.float32
    bf16 = mybir.dt.bfloat16
    fp16 = mybir.dt.float16
    x = nc.dram_tensor("x", [_B, _H, _W, _C], f32, kind="ExternalInput")
    y = nc.dram_tensor("y", [_B, _HO, _WO, _C], fp16, kind="ExternalOutput")

    # [256, 2, 16384]: xv[(b, ho), par, (w, c)] with par = h % 2
    xv = x.ap().rearrange("b (hh par) w c -> (b hh) par (w c)", par=2)
    # [256, 8192]
    yv = y.ap().rearrange("b i j c -> (b i) (j c)")

    relu = mybir.ActivationFunctionType.Relu
    square = mybir.ActivationFunctionType.Square
    ln_f = mybir.ActivationFunctionType.Ln
    exp_f = mybir.ActivationFunctionType.Exp
    add_op = mybir.AluOpType.add
    mult_op = mybir.AluOpType.mult

    # piece list: (group, col offset, width, starts-new-tile)
    pieces = []
    for tix in range(_NG * 4):
        g, base = tix // 4, (tix % 4) * _F
        if tix == 0:
            ws = [1024, 1024, 2048]
        elif tix in (1, 2):
            # half-tile pieces during pipeline ramp: relu can start on the
            # first half-load instead of waiting out the full 10us tile DMA
            ws = [2048, 2048]
        elif tix == _NG * 4 - 1:
            ws = [2048, 1024, 1024]
        else:
            ws = [_F]
        off = 0
        for j, w in enumerate(ws):
            pieces.append((g, base, base + off, w, j == 0, tix))
            off += w

    with tile.TileContext(nc) as tc:
        with (
            tc.tile_pool(name="io", bufs=2) as io,
            tc.tile_pool(name="rq", bufs=2) as rq,
            tc.tile_pool(name="t1", bufs=1) as t1,
            tc.tile_pool(name="t2", bufs=2) as t2,
            tc.tile_pool(name="t3", bufs=3) as t3,
            tc.tile_pool(name="ps", bufs=2, space="PSUM") as ps,
            tc.tile_pool(name="ot", bufs=3) as ot,
        ):
            # Warm the ACT function-table (~1.3us load) on dummy data so it
            # overlaps the first input DMA instead of delaying the first relu.
            warm0 = t1.tile([_NP, 8], f32, tag="warm0")
            warmb = t1.tile([_NP, 8], bf16, tag="warmb")
            warmh = t1.tile([_NP, 8], fp16, tag="warmh")
            ceps = t1.tile([_NP, 1], f32, tag="ceps")
            clnk = t1.tile([_NP, 1], f32, tag="clnk")
            nc.vector.memset(ceps[:], _EPS)
            nc.vector.memset(clnk[:], _LN_OSCALE)
            nc.vector.memset(warm0[:], 1.0)
            nc.scalar.activation(warmb[:], warm0[:], relu)
            nc.scalar.activation(warmb[:], warmb[:], square)
            nc.scalar.activation(warmh[:], warmb[:], ln_f, bias=ceps[:])
            nc.scalar.activation(warmb[:], warmh[:], exp_f, scale=-1.0, bias=clnk[:])

            cur = {}

            def stage1(p):
                g, tbase, c0, w, newt, tix = p
                wo, w2 = w // 2, 2 * w
                p0, p1 = g * _NP, (g + 1) * _NP
                if newt:
                    cur["EO"] = io.tile([_NP, 2 * _F], f32, tag="EO", name="EO")
                    cur["R"] = rq.tile([_NP, 2 * _F], bf16, tag="R", name="R")
                    cur["Q"] = rq.tile([_NP, 2 * _F], bf16, tag="Q", name="Q")
                EO, R, Q = cur["EO"], cur["R"], cur["Q"]
                a = 2 * (c0 - tbase)
                eov = EO[:, a:a + w2].rearrange("p (par f) -> p par f", par=2)
                nc.sync.dma_start(eov, xv[p0:p1, :, c0:c0 + w])

                sw = t1.tile([_NP, _F], bf16, tag="sw")
                ssw = t1.tile([_NP, _F], bf16, tag="ssw")
                if _DIV_MODE == "lnexp":
                    s = t2.tile([_NP, _F // 2], bf16, tag="s")
                else:
                    s = t2.tile([_NP, _F // 2], f32, tag="s")
                ss = t3.tile([_NP, _F // 2], bf16, tag="ss")

                def prs(tile_):
                    # piece view [128, 2, w/512, 2, 256]: (h, wgrp, wpar, c)
                    v = tile_[:, a:a + w2].rearrange(
                        "p (h w par c) -> p h w par c", h=2, par=2, c=_C
                    )
                    return v[:, :, :, 0, :], v[:, :, :, 1, :]

                def hp(tile_):
                    return tile_[:, :w].rearrange("p (h w c) -> p h w c", h=2, c=_C)

                # relu + downcast to bf16 in one ACT pass
                nc.scalar.activation(R[:, a:a + w2], EO[:, a:a + w2], relu)
                # squares split ACT/DVE for engine balance; the ACT part is
                # emitted before the DVE square so the same-tile write
                # ordering points DVE -> ACT-early, not ACT -> DVE-late.
                # lnexp mode: ACT also runs ln+exp, so it takes only 3/16;
                # recip mode: the divide is on DVE, ACT takes 7/8 -- except
                # at the edges: DVE idles during the ACT-bound warm-up, so
                # the first tile's squares all go to DVE; the run's tail is
                # all-DVE (divide chain), so the last tile's go to ACT.
                if _DIV_MODE == "lnexp":
                    qs = (w2 * 3) // 16
                elif tix == _NG * 4 - 1:
                    qs = w2
                elif tix == 0:
                    qs = 0
                else:
                    qs = (w2 * 7) // 8
                if qs:
                    nc.scalar.activation(Q[:, a:a + qs], R[:, a:a + qs], square)
                Re, Ro = prs(R)
                # w-pair adds for both h-rows in one bf16 op: sw = [sE | sO]
                nc.vector.tensor_add(hp(sw), Re, Ro)
                if _DIV_MODE == "lnexp":
                    # s = sE + sO (bf16 2x; the 0/0 guard rides the ln bias)
                    nc.vector.tensor_add(s[:, :wo], sw[:, :wo], sw[:, wo:w])
                else:
                    # s = (sE + eps) + sO in f32 for the fp32 reciprocal
                    nc.vector.scalar_tensor_tensor(
                        s[:, :wo], sw[:, :wo], _EPS, sw[:, wo:w],
                        op0=add_op, op1=add_op,
                    )
                if qs < w2:
                    nc.vector.tensor_mul(
                        Q[:, a + qs:a + w2], R[:, a + qs:a + w2], R[:, a + qs:a + w2]
                    )
                Qe, Qo = prs(Q)
                nc.vector.tensor_add(hp(ssw), Qe, Qo)
                nc.vector.tensor_add(ss[:, :wo], ssw[:, :wo], ssw[:, wo:w])
                return {"g": g, "c0": c0, "wo": wo, "s": s, "ss": ss}

            def stage2a(st):
                # t = 1024/s: ACT exp(-ln(s + eps) + ln1024) in lnexp mode,
                # DVE reciprocal (x1024 via the final mul? no: scale below)
                # in recip mode. t is f32 (bf16 t costs ~0.4% rel err) and
                # lives in PSUM, which is otherwise unused and leaves SBUF
                # room for deep buffers.
                wo = st["wo"]
                t = ps.tile([_NP, _F // 2], f32, tag="t", name="t")
                if _DIV_MODE == "lnexp":
                    L = t1.tile([_NP, _F // 2], fp16, tag="L")
                    nc.scalar.activation(
                        L[:, :wo], st["s"][:, :wo], ln_f, bias=ceps[:]
                    )
                    nc.scalar.activation(
                        t[:, :wo], L[:, :wo], exp_f, scale=-1.0, bias=clnk[:]
                    )
                else:
                    nc.vector.reciprocal_approx_fast(t[:, :wo], st["s"][:, :wo])
                st["t"] = t

            def stage2b(st):
                # DVE: o = 1024*ss*t -> fp16 ; store via the GpSimd queue.
                # lnexp mode: the 1024 is already inside t; recip mode folds
                # it into a scalar_tensor_tensor at the same cost.
                g, c0, wo = st["g"], st["c0"], st["wo"]
                p0, p1 = g * _NP, (g + 1) * _NP
                o = ot.tile([_NP, _F // 2], fp16, tag="o")
                if _DIV_MODE == "lnexp":
                    nc.vector.tensor_mul(
                        o[:, :wo], st["ss"][:, :wo], st["t"][:, :wo]
                    )
                else:
                    nc.vector.scalar_tensor_tensor(
                        o[:, :wo], st["ss"][:, :wo], 1024.0, st["t"][:, :wo],
                        op0=mult_op, op1=mult_op,
                    )
                nc.gpsimd.dma_start(yv[p0:p1, c0 // 2:c0 // 2 + wo], o[:, :wo])

            # 2-deep software pipeline: ln/exp run one piece behind the
            # reduction, mul/store two behind, so neither engine waits on
            # the other's mid-round output.
            hist = []
            for p in pieces:
                st = stage1(p)
                hist.append(st)
                if len(hist) >= 2:
                    stage2a(hist[-2])
                if len(hist) >= 3:
                    stage2b(hist[-3])
            stage2a(hist[-1])
            stage2b(hist[-2])
            stage2b(hist[-1])

    nc.compile()
    return nc


def _get_nc():
    if "nc" not in _CACHE:
        _CACHE["nc"] = _build_nc()
    return _CACHE["nc"]


def kernel(x: np.ndarray) -> np.ndarray:
    from concourse.bass_utils import run_bass_kernel_spmd

    nc = _get_nc()
    x = np.ascontiguousarray(np.asarray(x, dtype=np.float32))
    shards = np.split(x, 8, axis=0)
    in_maps = [{"x": s} for s in shards]
    res = run_bass_kernel_spmd(nc, in_maps, list(range(8)))
    out = np.concatenate([res.results[i]["y"] for i in range(8)], axis=0)
    return out.astype(np.float32) * np.float32(_OSCALE_INV)


# revision 26
# speedup vs baseline: 1.0190x; 1.0190x over previous
"""AcceptRejectPooling2D on 8 Trainium2 NeuronCores.

Reference semantics (per 2x2 window, stride 2, NHWC):
    r  = relu(x)
    s  = sum(r); ss = sum(r*r)
    out = ss / s   if s > 0 else 0

Sharding: pure data parallel over batch (64 -> 8 per core). Each core
processes x_local [8, 64, 64, 256] -> y_local [8, 32, 32, 256].

Layout per core: rows (b, h) of length W*C = 16384 floats. Output row
p = (b, ho) needs input rows 2p (even h) and 2p+1 (odd h). 256 output
rows = 2 partition groups of 128, each streamed as 4 tiles of 4096
row-columns ([128, 8192] f32 in SBUF).

Division via ACT tables: t = exp(-ln(s + eps) + ln(1024)) = 1024/s.
This keeps the whole reduction 16-bit on the DVE (2x packed mode for
every add and the final mul) and moves the divide to the scalar engine
whose Ln/Exp run at full rate from the same function table as
Relu/Square. ln output is fp16 (bf16's 8-bit mantissa costs ~3% in
exp(ln) round-trip at |ln|~9); t is bf16 (needs e+30 range for the
eps-guarded zero windows, where ss=0 makes out exactly 0). The output
is stored as fp16 scaled by 1024 (so values down to 6e-8 stay in fp16
normals) and unscaled on the host.

Engine balance per 4096-col chunk: ACT = relu + ln + exp + 1/8 of the
square; DVE = pair-adds + 7/8 of the square + final mul. Both ~12us.
A 2-deep software pipeline (ln/exp one chunk behind, mul/store two
behind) keeps the cross-engine handoffs off the critical path. First
and last tiles are sub-chunked so the pipe fills/drains fine-grained.
"""

import sys

if "/opt/trn_rl_repo" not in sys.path:
    sys.path.insert(0, "/opt/trn_rl_repo")

import math

import numpy as np

_B, _H, _W, _C = 8, 64, 64, 256  # per-core shard
_HO, _WO = _H // 2, _W // 2
_NP = 128                         # SBUF partitions
_F = 4096                         # row-columns per tile
_NG = (_B * _HO) // _NP           # partition groups (2)
_EPS = 1e-30
_LN_OSCALE = math.log(1024.0)     # out stored as 1024*(ss/s), fp16
_OSCALE_INV = 1.0 / 1024.0
# "lnexp": t = exp(-ln(s)+k) on ACT (faster, ~1.7e-2 rel err)
# "recip": t = 1024/s via DVE reciprocal_approx_fast (~1.3e-2 rel err)
_DIV_MODE = "recip"

_CACHE = {}


def _pin_act_table(bacc, mybir):
    """Route every activation to natural_log_exp_and_others (which holds
    Relu, Square, Ln AND Exp) so the kernel needs exactly one ACT
    function-table load. The compiler's per-instruction greedy set choice
    otherwise alternates sets (~2.7us reload each). Only the in-memory
    choice list is edited; set ids / loaded table bytes are unchanged.
    """
    if getattr(bacc, "_arp_act_pin", False):
        return
    AF = mybir.ActivationFunctionType
    pin = {AF.Relu, AF.Square, AF.Ln, AF.Exp}
    orig = bacc.get_activation_tables

    def pinned(arch):
        return {
            name: (fns if name == "natural_log_exp_and_others" else fns - pin)
            for name, fns in orig(arch).items()
        }

    bacc.get_activation_tables = pinned
    bacc._arp_act_pin = True


def _build_nc():
    import concourse.bacc as bacc
    import concourse.tile as tile
    from concourse import mybir

    _pin_act_table(bacc, mybir)
    nc = bacc.Bacc("TRN2", target_bir_lowering=False, debug=False, num_devices=8)
    f32 = mybir.dt.float32
    bf16 = mybir.dt.bfloat16
    fp16 = mybir.dt.float16
    x = nc.dram_tensor("x", [_B, _H, _W, _C], f32, kind="ExternalInput")
    y = nc.dram_tensor("y", [_B, _HO, _WO, _C], fp16, kind="ExternalOutput")

    # [256, 2, 16384]: xv[(b, ho), par, (w, c)] with par = h % 2
    xv = x.ap().rearrange("b (hh par) w c -> (b hh) par (w c)", par=2)
    # [256, 8192]
    yv = y.ap().rearrange("b i j c -> (b i) (j c)")

    relu = mybir.ActivationFunctionType.Relu
    square = mybir.ActivationFunctionType.Square
    ln_f = mybir.ActivationFunctionType.Ln
    exp_f = mybir.ActivationFunctionType.Exp
    add_op = mybir.AluOpType.add
    mult_op = mybir.AluOpType.mult

    # piece list: (group, col offset, width, starts-new-tile)
    pieces = []
    for tix in range(_NG * 4):
        g, base = tix // 4, (tix % 4) * _F
        if tix == 0:
            ws = [1024, 1024, 2048]
        elif tix in (1, 2):
            # half-tile pieces during pipeline ramp: relu can start on the
            # first half-load instead of waiting out the full 10us tile DMA
            ws = [2048, 2048]
        elif tix == _NG * 4 - 1:
            ws = [2048, 1024, 1024]
        else:
            ws = [_F]
        off = 0
        for j, w in enumerate(ws):
            pieces.append((g, base, base + off, w, j == 0, tix))
            off += w

    with tile.TileContext(nc) as tc:
        with (
            tc.tile_pool(name="io", bufs=2) as io,
            tc.tile_pool(name="rq", bufs=2) as rq,
            tc.tile_pool(name="t1", bufs=1) as t1,
            tc.tile_pool(name="t2", bufs=2) as t2,
            tc.tile_pool(name="t3", bufs=3) as t3,
            tc.tile_pool(name="ps", bufs=2, space="PSUM") as ps,
            tc.tile_pool(name="ot", bufs=3) as ot,
        ):
            # Warm the ACT function-table (~1.3us load) on dummy data so it
            # overlaps the first input DMA instead of delaying the first relu.
            warm0 = t1.tile([_NP, 8], f32, tag="warm0")
            warmb = t1.tile([_NP, 8], bf16, tag="warmb")
            warmh = t1.tile([_NP, 8], fp16, tag="warmh")
            ceps = t1.tile([_NP, 1], f32, tag="ceps")
            clnk = t1.tile([_NP, 1], f32, tag="clnk")
            nc.vector.memset(ceps[:], _EPS)
            nc.vector.memset(clnk[:], _LN_OSCALE)
            nc.vector.memset(warm0[:], 1.0)
            nc.scalar.activation(warmb[:], warm0[:], relu)
            nc.scalar.activation(warmb[:], warmb[:], square)
            nc.scalar.activation(warmh[:], warmb[:], ln_f, bias=ceps[:])
            nc.scalar.activation(warmb[:], warmh[:], exp_f, scale=-1.0, bias=clnk[:])

            cur = {}

            def stage1(p):
                g, tbase, c0, w, newt, tix = p
                wo, w2 = w // 2, 2 * w
                p0, p1 = g * _NP, (g + 1) * _NP
                if newt:
                    cur["EO"] = io.tile([_NP, 2 * _F], f32, tag="EO", name="EO")
                    cur["R"] = rq.tile([_NP, 2 * _F], bf16, tag="R", name="R")
                    cur["Q"] = rq.tile([_NP, 2 * _F], bf16, tag="Q", name="Q")
                EO, R, Q = cur["EO"], cur["R"], cur["Q"]
                a = 2 * (c0 - tbase)
                eov = EO[:, a:a + w2].rearrange("p (par f) -> p par f", par=2)
                nc.sync.dma_start(eov, xv[p0:p1, :, c0:c0 + w])

                sw = t1.tile([_NP, _F], bf16, tag="sw")
                ssw = t1.tile([_NP, _F], bf16, tag="ssw")
                if _DIV_MODE == "lnexp":
                    s = t2.tile([_NP, _F // 2], bf16, tag="s")
                else:
                    s = t2.tile([_NP, _F // 2], f32, tag="s")
                ss = t3.tile([_NP, _F // 2], bf16, tag="ss")

                def prs(tile_):
                    # piece view [128, 2, w/512, 2, 256]: (h, wgrp, wpar, c)
                    v = tile_[:, a:a + w2].rearrange(
                        "p (h w par c) -> p h w par c", h=2, par=2, c=_C
                    )
                    return v[:, :, :, 0, :], v[:, :, :, 1, :]

                def hp(tile_):
                    return tile_[:, :w].rearrange("p (h w c) -> p h w c", h=2, c=_C)

                # relu + downcast to bf16 in one ACT pass
                nc.scalar.activation(R[:, a:a + w2], EO[:, a:a + w2], relu)
                # squares split ACT/DVE for engine balance; the ACT part is
                # emitted before the DVE square so the same-tile write
                # ordering points DVE -> ACT-early, not ACT -> DVE-late.
                # lnexp mode: ACT also runs ln+exp, so it takes only 3/16;
                # recip mode: the divide is on DVE, ACT takes 7/8 -- except
                # at the edges: DVE idles during the ACT-bound warm-up, so
                # the first tile's squares all go to DVE; the run's tail is
                # all-DVE (divide chain), so the last tile's go to ACT.
                if _DIV_MODE == "lnexp":
                    qs = (w2 * 3) // 16
                elif tix == _NG * 4 - 1:
                    qs = w2
                else:
                    qs = (w2 * 13) // 16
                if qs:
                    nc.scalar.activation(Q[:, a:a + qs], R[:, a:a + qs], square)
                Re, Ro = prs(R)
                # w-pair adds for both h-rows in one bf16 op: sw = [sE | sO]
                nc.vector.tensor_add(hp(sw), Re, Ro)
                if _DIV_MODE == "lnexp":
                    # s = sE + sO (bf16 2x; the 0/0 guard rides the ln bias)
                    nc.vector.tensor_add(s[:, :wo], sw[:, :wo], sw[:, wo:w])
                else:
                    # s = (sE + eps) + sO in f32 for the fp32 reciprocal
                    nc.vector.scalar_tensor_tensor(
                        s[:, :wo], sw[:, :wo], _EPS, sw[:, wo:w],
                        op0=add_op, op1=add_op,
                    )
                if qs < w2:
                    nc.vector.tensor_mul(
                        Q[:, a + qs:a + w2], R[:, a + qs:a + w2], R[:, a + qs:a + w2]
                    )
                Qe, Qo = prs(Q)
                nc.vector.tensor_add(hp(ssw), Qe, Qo)
                nc.vector.tensor_add(ss[:, :wo], ssw[:, :wo], ssw[:, wo:w])
                return {"g": g, "c0": c0, "wo": wo, "s": s, "ss": ss}

            def stage2a(st):
                # t = 1024/s: ACT exp(-ln(s + eps) + ln1024) in lnexp mode,
                # DVE reciprocal (x1024 via the final mul? no: scale below)
                # in recip mode. t is f32 (bf16 t costs ~0.4% rel err) and
                # lives in PSUM, which is otherwise unused and leaves SBUF
                # room for deep buffers.
                wo = st["wo"]
                t = ps.tile([_NP, _F // 2], f32, tag="t", name="t")
                if _DIV_MODE == "lnexp":
                    L = t1.tile([_NP, _F // 2], fp16, tag="L")
                    nc.scalar.activation(
                        L[:, :wo], st["s"][:, :wo], ln_f, bias=ceps[:]
                    )
                    nc.scalar.activation(
                        t[:, :wo], L[:, :wo], exp_f, scale=-1.0, bias=clnk[:]
                    )
                else:
                    nc.vector.reciprocal_approx_fast(t[:, :wo], st["s"][:, :wo])
                st["t"] = t

            def stage2b(st):
                # DVE: o = 1024*ss*t -> fp16 ; store via the GpSimd queue.
                # lnexp mode: the 1024 is already inside t; recip mode folds
                # it into a scalar_tensor_tensor at the same cost.
                g, c0, wo = st["g"], st["c0"], st["wo"]
                p0, p1 = g * _NP, (g + 1) * _NP
                o = ot.tile([_NP, _F // 2], fp16, tag="o")
                if _DIV_MODE == "lnexp":
                    nc.vector.tensor_mul(
                        o[:, :wo], st["ss"][:, :wo], st["t"][:, :wo]
                    )
                else:
                    nc.vector.scalar_tensor_tensor(
                        o[:, :wo], st["ss"][:, :wo], 1024.0, st["t"][:, :wo],
                        op0=mult_op, op1=mult_op,
                    )
                nc.gpsimd.dma_start(yv[p0:p1, c0 // 2:c0 // 2 + wo], o[:, :wo])

            # 2-deep software pipeline: ln/exp run one piece behind the
            # reduction, mul/store two behind, so neither engine waits on
            # the other's mid-round output.
            hist = []
            for p in pieces:
                st = stage1(p)
                hist.append(st)
                if len(hist) >= 2:
                    stage2a(hist[-2])
                if len(hist) >= 3:
                    stage2b(hist[-3])
            stage2a(hist[-1])
            stage2b(hist[-2])
            stage2b(hist[-1])

    nc.compile()
    return nc


def _get_nc():
    if "nc" not in _CACHE:
        _CACHE["nc"] = _build_nc()
    return _CACHE["nc"]


def kernel(x: np.ndarray) -> np.ndarray:
    from concourse.bass_utils import run_bass_kernel_spmd

    nc = _get_nc()
    x = np.ascontiguousarray(np.asarray(x, dtype=np.float32))
    shards = np.split(x, 8, axis=0)
    in_maps = [{"x": s} for s in shards]
    res = run_bass_kernel_spmd(nc, in_maps, list(range(8)))
    out = np.concatenate([res.results[i]["y"] for i in range(8)], axis=0)
    return out.astype(np.float32) * np.float32(_OSCALE_INV)


# revision 27
# speedup vs baseline: 1.0818x; 1.0616x over previous
"""AcceptRejectPooling2D on 8 Trainium2 NeuronCores.

Reference semantics (per 2x2 window, stride 2, NHWC):
    r  = relu(x)
    s  = sum(r); ss = sum(r*r)
    out = ss / s   if s > 0 else 0

Sharding: pure data parallel over batch (64 -> 8 per core). Each core
processes x_local [8, 64, 64, 256] -> y_local [8, 32, 32, 256].

Layout per core: rows (b, h) of length W*C = 16384 floats. Output row
p = (b, ho) needs input rows 2p (even h) and 2p+1 (odd h). 256 output
rows = 2 partition groups of 128, each streamed as 4 tiles of 4096
row-columns ([128, 8192] f32 in SBUF).

Division via ACT tables: t = exp(-ln(s + eps) + ln(1024)) = 1024/s.
This keeps the whole reduction 16-bit on the DVE (2x packed mode for
every add and the final mul) and moves the divide to the scalar engine
whose Ln/Exp run at full rate from the same function table as
Relu/Square. ln output is fp16 (bf16's 8-bit mantissa costs ~3% in
exp(ln) round-trip at |ln|~9); t is bf16 (needs e+30 range for the
eps-guarded zero windows, where ss=0 makes out exactly 0). The output
is stored as fp16 scaled by 1024 (so values down to 6e-8 stay in fp16
normals) and unscaled on the host.

Engine balance per 4096-col chunk: ACT = relu + ln + exp + 1/8 of the
square; DVE = pair-adds + 7/8 of the square + final mul. Both ~12us.
A 2-deep software pipeline (ln/exp one chunk behind, mul/store two
behind) keeps the cross-engine handoffs off the critical path. First
and last tiles are sub-chunked so the pipe fills/drains fine-grained.
"""

import sys

if "/opt/trn_rl_repo" not in sys.path:
    sys.path.insert(0, "/opt/trn_rl_repo")

import math

import numpy as np

_B, _H, _W, _C = 8, 64, 64, 256  # per-core shard
_HO, _WO = _H // 2, _W // 2
_NP = 128                         # SBUF partitions
_F = 4096                         # row-columns per tile
_NG = (_B * _HO) // _NP           # partition groups (2)
_EPS = 1e-30
_LN_OSCALE = math.log(1024.0)     # out stored as 1024*(ss/s), fp16
_OSCALE_INV = 1.0 / 1024.0
# "lnexp": t = exp(-ln(s)+k) on ACT (faster, ~1.7e-2 rel err)
# "recip": t = 1024/s via DVE reciprocal_approx_fast (~1.3e-2 rel err)
_DIV_MODE = "recip"

_CACHE = {}


def _pin_act_table(bacc, mybir):
    """Route every activation to natural_log_exp_and_others (which holds
    Relu, Square, Ln AND Exp) so the kernel needs exactly one ACT
    function-table load. The compiler's per-instruction greedy set choice
    otherwise alternates sets (~2.7us reload each). Only the in-memory
    choice list is edited; set ids / loaded table bytes are unchanged.
    """
    if getattr(bacc, "_arp_act_pin", False):
        return
    AF = mybir.ActivationFunctionType
    pin = {AF.Relu, AF.Square, AF.Ln, AF.Exp}
    orig = bacc.get_activation_tables

    def pinned(arch):
        return {
            name: (fns if name == "natural_log_exp_and_others" else fns - pin)
            for name, fns in orig(arch).items()
        }

    bacc.get_activation_tables = pinned
    bacc._arp_act_pin = True


def _build_nc():
    import concourse.bacc as bacc
    import concourse.tile as tile
    from concourse import mybir

    _pin_act_table(bacc, mybir)
    nc = bacc.Bacc("TRN2", target_bir_lowering=False, debug=False, num_devices=8)
    f32 = mybir.dt.float32
    bf16 = mybir.dt.bfloat16
    fp16 = mybir.dt.float16
    x = nc.dram_tensor("x", [_B, _H, _W, _C], f32, kind="ExternalInput")
    y = nc.dram_tensor("y", [_B, _HO, _WO, _C], fp16, kind="ExternalOutput")

    # [256, 2, 16384]: xv[(b, ho), par, (w, c)] with par = h % 2
    xv = x.ap().rearrange("b (hh par) w c -> (b hh) par (w c)", par=2)
    # [256, 8192]
    yv = y.ap().rearrange("b i j c -> (b i) (j c)")

    relu = mybir.ActivationFunctionType.Relu
    square = mybir.ActivationFunctionType.Square
    ln_f = mybir.ActivationFunctionType.Ln
    exp_f = mybir.ActivationFunctionType.Exp
    add_op = mybir.AluOpType.add
    mult_op = mybir.AluOpType.mult

    # piece list: (group, col offset, width, starts-new-tile)
    pieces = []
    for tix in range(_NG * 4):
        g, base = tix // 4, (tix % 4) * _F
        if tix == 0:
            ws = [1024, 1024, 2048]
        elif tix in (1, 2):
            # half-tile pieces during pipeline ramp: relu can start on the
            # first half-load instead of waiting out the full 10us tile DMA
            ws = [2048, 2048]
        elif tix == _NG * 4 - 1:
            ws = [2048, 1024, 1024]
        else:
            ws = [_F]
        off = 0
        for j, w in enumerate(ws):
            pieces.append((g, base, base + off, w, j == 0, tix))
            off += w

    with tile.TileContext(nc) as tc:
        with (
            tc.tile_pool(name="io", bufs=2) as io,
            tc.tile_pool(name="rq", bufs=2) as rq,
            tc.tile_pool(name="t1", bufs=1) as t1,
            tc.tile_pool(name="t2", bufs=2) as t2,
            tc.tile_pool(name="t3", bufs=3) as t3,
            tc.tile_pool(name="ps", bufs=2, space="PSUM") as ps,
            tc.tile_pool(name="ot", bufs=3) as ot,
        ):
            # Warm the ACT function-table (~1.3us load) on dummy data so it
            # overlaps the first input DMA instead of delaying the first relu.
            warm0 = t1.tile([_NP, 8], f32, tag="warm0")
            warmb = t1.tile([_NP, 8], bf16, tag="warmb")
            warmh = t1.tile([_NP, 8], fp16, tag="warmh")
            ceps = t1.tile([_NP, 1], f32, tag="ceps")
            clnk = t1.tile([_NP, 1], f32, tag="clnk")
            nc.vector.memset(ceps[:], _EPS)
            nc.vector.memset(clnk[:], _LN_OSCALE)
            nc.vector.memset(warm0[:], 1.0)
            nc.scalar.activation(warmb[:], warm0[:], relu)
            nc.scalar.activation(warmb[:], warmb[:], square)
            nc.scalar.activation(warmh[:], warmb[:], ln_f, bias=ceps[:])
            nc.scalar.activation(warmb[:], warmh[:], exp_f, scale=-1.0, bias=clnk[:])

            cur = {}

            def stage1(p):
                g, tbase, c0, w, newt, tix = p
                wo, w2 = w // 2, 2 * w
                p0, p1 = g * _NP, (g + 1) * _NP
                if newt:
                    cur["EO"] = io.tile([_NP, 2 * _F], f32, tag="EO", name="EO")
                    cur["R"] = rq.tile([_NP, 2 * _F], bf16, tag="R", name="R")
                    cur["Q"] = rq.tile([_NP, 2 * _F], bf16, tag="Q", name="Q")
                EO, R, Q = cur["EO"], cur["R"], cur["Q"]
                a = 2 * (c0 - tbase)
                eov = EO[:, a:a + w2].rearrange("p (par f) -> p par f", par=2)
                nc.sync.dma_start(eov, xv[p0:p1, :, c0:c0 + w])

                sw = t1.tile([_NP, _F], bf16, tag="sw")
                ssw = t1.tile([_NP, _F], bf16, tag="ssw")
                if _DIV_MODE == "lnexp":
                    s = t2.tile([_NP, _F // 2], bf16, tag="s")
                else:
                    s = t2.tile([_NP, _F // 2], f32, tag="s")
                ss = t3.tile([_NP, _F // 2], bf16, tag="ss")

                def prs(tile_):
                    # piece view [128, 2, w/512, 2, 256]: (h, wgrp, wpar, c)
                    v = tile_[:, a:a + w2].rearrange(
                        "p (h w par c) -> p h w par c", h=2, par=2, c=_C
                    )
                    return v[:, :, :, 0, :], v[:, :, :, 1, :]

                def hp(tile_):
                    return tile_[:, :w].rearrange("p (h w c) -> p h w c", h=2, c=_C)

                # relu + downcast to bf16 in one ACT pass
                nc.scalar.activation(R[:, a:a + w2], EO[:, a:a + w2], relu)
                # squares split ACT/DVE for engine balance; the ACT part is
                # emitted before the DVE square so the same-tile write
                # ordering points DVE -> ACT-early, not ACT -> DVE-late.
                # lnexp mode: ACT also runs ln+exp, so it takes only 3/16;
                # recip mode: the divide is on DVE, ACT takes 7/8 -- except
                # at the edges: DVE idles during the ACT-bound warm-up, so
                # the first tile's squares all go to DVE; the run's tail is
                # all-DVE (divide chain), so the last tile's go to ACT.
                if _DIV_MODE == "lnexp":
                    qs = (w2 * 3) // 16
                else:
                    qs = (w2 * 13) // 16
                if qs:
                    nc.scalar.activation(Q[:, a:a + qs], R[:, a:a + qs], square)
                Re, Ro = prs(R)
                # w-pair adds for both h-rows in one bf16 op: sw = [sE | sO]
                nc.vector.tensor_add(hp(sw), Re, Ro)
                if _DIV_MODE == "lnexp":
                    # s = sE + sO (bf16 2x; the 0/0 guard rides the ln bias)
                    nc.vector.tensor_add(s[:, :wo], sw[:, :wo], sw[:, wo:w])
                else:
                    # s = (sE + eps) + sO in f32 for the fp32 reciprocal
                    nc.vector.scalar_tensor_tensor(
                        s[:, :wo], sw[:, :wo], _EPS, sw[:, wo:w],
                        op0=add_op, op1=add_op,
                    )
                if qs < w2:
                    nc.vector.tensor_mul(
                        Q[:, a + qs:a + w2], R[:, a + qs:a + w2], R[:, a + qs:a + w2]
                    )
                Qe, Qo = prs(Q)
                nc.vector.tensor_add(hp(ssw), Qe, Qo)
                nc.vector.tensor_add(ss[:, :wo], ssw[:, :wo], ssw[:, wo:w])
                return {"g": g, "c0": c0, "wo": wo, "s": s, "ss": ss}

            def stage2a(st):
                # t = 1024/s: ACT exp(-ln(s + eps) + ln1024) in lnexp mode,
                # DVE reciprocal (x1024 via the final mul? no: scale below)
                # in recip mode. t is f32 (bf16 t costs ~0.4% rel err) and
                # lives in PSUM, which is otherwise unused and leaves SBUF
                # room for deep buffers.
                wo = st["wo"]
                t = ps.tile([_NP, _F // 2], f32, tag="t", name="t")
                if _DIV_MODE == "lnexp":
                    L = t1.tile([_NP, _F // 2], fp16, tag="L")
                    nc.scalar.activation(
                        L[:, :wo], st["s"][:, :wo], ln_f, bias=ceps[:]
                    )
                    nc.scalar.activation(
                        t[:, :wo], L[:, :wo], exp_f, scale=-1.0, bias=clnk[:]
                    )
                else:
                    nc.vector.reciprocal_approx_fast(t[:, :wo], st["s"][:, :wo])
                st["t"] = t

            def stage2b(st):
                # DVE: o = 1024*ss*t -> fp16 ; store via the GpSimd queue.
                # lnexp mode: the 1024 is already inside t; recip mode folds
                # it into a scalar_tensor_tensor at the same cost.
                g, c0, wo = st["g"], st["c0"], st["wo"]
                p0, p1 = g * _NP, (g + 1) * _NP
                o = ot.tile([_NP, _F // 2], fp16, tag="o")
                if _DIV_MODE == "lnexp":
                    nc.vector.tensor_mul(
                        o[:, :wo], st["ss"][:, :wo], st["t"][:, :wo]
                    )
                else:
                    nc.vector.scalar_tensor_tensor(
                        o[:, :wo], st["ss"][:, :wo], 1024.0, st["t"][:, :wo],
                        op0=mult_op, op1=mult_op,
                    )
                nc.gpsimd.dma_start(yv[p0:p1, c0 // 2:c0 // 2 + wo], o[:, :wo])

            # 2-deep software pipeline: ln/exp run one piece behind the
            # reduction, mul/store two behind, so neither engine waits on
            # the other's mid-round output.
            hist = []
            for p in pieces:
                st = stage1(p)
                hist.append(st)
                if len(hist) >= 2:
                    stage2a(hist[-2])
                if len(hist) >= 3:
                    stage2b(hist[-3])
            stage2a(hist[-1])
            stage2b(hist[-2])
            stage2b(hist[-1])

    nc.compile()
    return nc


def _get_nc():
    if "nc" not in _CACHE:
        _CACHE["nc"] = _build_nc()
    return _CACHE["nc"]


def kernel(x: np.ndarray) -> np.ndarray:
    from concourse.bass_utils import run_bass_kernel_spmd

    nc = _get_nc()
    x = np.ascontiguousarray(np.asarray(x, dtype=np.float32))
    shards = np.split(x, 8, axis=0)
    in_maps = [{"x": s} for s in shards]
    res = run_bass_kernel_spmd(nc, in_maps, list(range(8)))
    out = np.concatenate([res.results[i]["y"] for i in range(8)], axis=0)
    return out.astype(np.float32) * np.float32(_OSCALE_INV)
